# revision 21
# baseline (speedup 1.0000x reference)
"""ArcFace loss kernel for Trainium2, class-sharded across 8 NeuronCores.

Strategy (vocab/tensor parallel per the module's own sharding):
  - Shard the class axis of `weights` (100000 classes -> 8 x 12800, zero-padded).
  - Host sends x-hat (normalized x) scaled by 8 in fp8e4m3 and raw w in bf16.
  - Each core normalizes its weight shard on device:
      q = w*w (split DVE / GPSIMD to balance engine load),
      n2 per class via one-hot-column matmuls that compact each chunk's
      partition-reduction onto psum row 32*m (so Ln/Exp over a whole 4-chunk
      group cost one 512-wide ACT op each instead of a broadcast pass),
      rw = 8/sqrt(n2) = exp(-0.5*ln(n2) + ln 8) on rows {0,32,64,96},
      rw broadcast to all 128 partitions by a K=1 ones matmul into PSUM,
      wn = w * rw cast to fp8 on DVE.
  - Main logits: DoubleRow fp8 matmul dot[b, c] = (8 x-hat).(8 w-hat) = 64*cos,
    then ScalarE exp(dot - 64) with accum_out producing per-row exp-sums.
  - Host: sum the 8 partial exp-sums (f64), fix up the 512 target-class
    entries with the ArcFace margin, take the mean cross-entropy.  A fixed
    shift of -64 (= -S, since cos <= 1) replaces the usual running max.

  - The norm pipeline is emitted one group ahead of the consuming matmul
    supers so Ln/Exp enter the ACT FIFO before the main exps that would
    otherwise delay them (DVE stalls on wn without this).

Ln+Exp share one ACT table set (see _fix_act_tables): no table reloads.
Measured (8 NeuronCores, trn2): relative error vs reference 1.1e-04;
per-core HW time ~134us repeat-loop-measured (~125us single-shot after
subtracting the measured 8.5us For_i back-edge), vs 155us for the prior
all-bf16 baseline.  Engine budget per core (cost model + microbenches):
ScalarE exp+norm ~79us, DVE q+wn ~81us, GPSIMD q ~82us, PE matmuls ~72us.
"""

import math

import ml_dtypes
import numpy as np

# Problem constants (hardcoded per contract; kernel.py must be self-contained).
B = 512  # batch
D = 512  # feature dim
C = 100000  # classes
S = 64.0
MARGIN = 0.5
COS_M = math.cos(MARGIN)
SIN_M = math.sin(MARGIN)
TH = math.cos(math.pi - MARGIN)
MM = math.sin(math.pi - MARGIN) * MARGIN

NCORES = 8
CH = 512  # classes per chunk (one PSUM bank of fp32)
NCH = 25  # chunks per core
CSH = CH * NCH  # 12800 padded classes per core
CPAD = CSH * NCORES  # 102400
KB = D // 128  # 4 contraction blocks
NB = B // 128  # 4 batch blocks
SHIFT = 64.0  # fixed logsumexp shift (logits = S*cos <= 64)
SUP = 3  # chunks per main-matmul super (PSUM banks per pm tile)
NSUP = (NCH + SUP - 1) // SUP
F8SCALE = 8.0  # power-of-2 scale keeping fp8 values normal-range

# norm-group chunk ranges (first group small for fast pipeline fill)
GROUPS = [(0, 2), (2, 5), (5, 8), (8, 11), (11, 14), (14, 17), (17, 20), (20, 23), (23, 25)]
# chunks whose q = w*w runs on DVE (rest on GPSIMD, balancing engine load)
Q_DVE = {0, 1, 2, 3, 4, 23, 24}

_CACHE = {}


def _fix_act_tables():
    """Make both Exp and Ln resolve to the one table set containing both.

    bass picks the first act-function set containing a needed function; by
    default Exp -> 'exp_and_others' and Ln -> 'natural_log' which thrashes the
    ACT table RAMs (~2.7us per reload).  Blank those two sets in the cached
    map (same dict object is returned every call) so both functions resolve
    to 'natural_log_exp_and_others'.
    """
    import concourse.hw_specs as hw_specs

    tables = hw_specs.get_activation_tables("gen3")
    for name in ("exp_and_others", "natural_log"):
        if name in tables and "natural_log_exp_and_others" in tables:
            tables[name].clear()


def _build_nc(repeat=1):
    import concourse.bass as bass
    import concourse.tile as tile
    from concourse import bacc, mybir

    _fix_act_tables()
    nc = bacc.Bacc(
        "TRN2",
        target_bir_lowering=False,
        debug=False,
        enable_asserts=False,
        num_devices=NCORES,
    )
    f8 = mybir.dt.float8e4
    bf16 = mybir.dt.bfloat16
    f32 = mybir.dt.float32
    DR = mybir.MatmulPerfMode.DoubleRow

    xnt = nc.dram_tensor("xnt", [D, B], f8, kind="ExternalInput").ap()
    wt = nc.dram_tensor("wt", [128, NCH, KB, CH], bf16, kind="ExternalInput").ap()
    s_out = nc.dram_tensor("s_out", [NB, 128], f32, kind="ExternalOutput").ap()

    from contextlib import ExitStack, nullcontext

    with tile.TileContext(nc) as tc, ExitStack() as ctx:
        singles = ctx.enter_context(tc.tile_pool(name="singles", bufs=1))
        wpool = ctx.enter_context(tc.tile_pool(name="wpool", bufs=9))
        qpool = ctx.enter_context(tc.tile_pool(name="qpool", bufs=8))
        lpool = ctx.enter_context(tc.tile_pool(name="lpool", bufs=3))
        rpool = ctx.enter_context(tc.tile_pool(name="rpool", bufs=3))
        escrp = ctx.enter_context(tc.tile_pool(name="escr", bufs=4))
        wnpool = ctx.enter_context(tc.tile_pool(name="wnpool", bufs=1))
        psp = ctx.enter_context(tc.tile_pool(name="psp", bufs=2, space="PSUM"))
        psm = ctx.enter_context(tc.tile_pool(name="psm", bufs=2, space="PSUM"))

        hint = (
            mybir.EngineType.PE,
            mybir.EngineType.Activation,
            mybir.EngineType.DVE,
            mybir.EngineType.Pool,
            mybir.EngineType.SP,
        )
        ctx.enter_context(
            tc.For_i(0, repeat, 1, hint_engines=hint) if repeat > 1 else nullcontext()
        )

        # x (stationary operand of every main matmul): [p, k, b], b-contiguous.
        xs = singles.tile([128, KB, B], f8)
        nc.sync.dma_start(out=xs[:], in_=xnt.rearrange("(k p) b -> p k b", p=128))

        ones32 = singles.tile([128, 128], bf16)
        nc.vector.memset(ones32[:], 1.0)
        # e_ms[m]: all-zero except column 32m = 1; as matmul lhsT it routes a
        # chunk's partition-sum onto psum row 32m.
        e_ms = []
        for m in range(4):
            e = singles.tile([128, 128], bf16, name=f"em{m}")
            nc.vector.memset(e[:], 0.0)
            nc.vector.memset(e[:, 32 * m : 32 * m + 1], 1.0)
            e_ms.append(e)

        eps_b = singles.tile([128, 1], f32)
        nc.vector.memset(eps_b[:], 1e-12)
        l8_b = singles.tile([128, 1], f32)
        nc.vector.memset(l8_b[:], math.log(F8SCALE))
        nshift_b = singles.tile([128, 1], f32)
        nc.vector.memset(nshift_b[:], -SHIFT)

        # accum_out landing area: one f32 scalar per (batch row, super).
        s_parts = singles.tile([128, NB * NSUP], f32)

        # Per-chunk resident normalized fp8 weights.
        wns = [
            wnpool.tile([128, KB, CH], f8, tag=f"wn{j}", name=f"wn{j}")
            for j in range(NCH)
        ]
        wts = {}

        supers = [list(range(c0, min(c0 + SUP, NCH))) for c0 in range(0, NCH, SUP)]

        emitted_group = 0
        wn_ready = 0
        for si, sup in enumerate(supers):
            while emitted_group < len(GROUPS) and wn_ready < min(NCH, sup[-1] + 1 + 8):
                g = emitted_group
                j0, j1 = GROUPS[g]
                pn = psp.tile([128, CH], f32, tag="ps", name=f"pn{g}")
                for m, j in enumerate(range(j0, j1)):
                    wtile = wpool.tile([128, KB, CH], bf16, tag="wt", name=f"wt{j}")
                    nc.sync.dma_start(out=wtile[:], in_=wt[:, j, :, :])
                    wts[j] = wtile
                    q = qpool.tile([128, KB, CH], bf16, tag="q", name=f"q{j}")
                    if j in Q_DVE:
                        nc.vector.tensor_mul(q[:], wtile[:], wtile[:])
                    else:
                        qf = q[:].rearrange("p k c -> p (k c)")
                        wf = wtile[:].rearrange("p k c -> p (k c)")
                        nc.gpsimd.tensor_mul(qf, wf, wf)
                    # n2 of chunk j -> psum row 32m (all other rows += 0)
                    for k in range(KB):
                        nc.tensor.matmul(
                            pn[:],
                            lhsT=e_ms[m][:],
                            rhs=q[:, k, :],
                            start=(m == 0 and k == 0),
                            stop=(m == j1 - j0 - 1 and k == KB - 1),
                        )
                lnt = lpool.tile([128, CH], f32, tag="lnt", name=f"lnt{g}")
                nc.scalar.activation(
                    lnt[:],
                    pn[:],
                    mybir.ActivationFunctionType.Ln,
                    bias=eps_b[:],
                    scale=1.0,
                )
                # rw = 8*rsqrt(n2) on rows {0,32,64,96}; junk rows harmless.
                rwg = rpool.tile([128, CH], bf16, tag="rw", name=f"rw{g}")
                nc.scalar.activation(
                    rwg[:],
                    lnt[:],
                    mybir.ActivationFunctionType.Exp,
                    bias=l8_b[:],
                    scale=-0.5,
                )
                for m, j in enumerate(range(j0, j1)):
                    # broadcast row 32m to all 128 partitions: ones outer rw
                    rb = psp.tile([128, CH], f32, tag="ps", name=f"rb{j}")
                    nc.tensor.matmul(
                        rb[:],
                        lhsT=ones32[32 * m : 32 * m + 1, :],
                        rhs=rwg[32 * m : 32 * m + 1, :],
                        start=True,
                        stop=True,
                        tile_position=(32 * m, 0),
                    )
                    rb_b = bass.AP(
                        tensor=rb.tensor,
                        offset=rb.offset,
                        ap=[rb.ap[0], [0, KB], rb.ap[-1]],
                    )
                    # wn = w * rw -> fp8 (rw read straight from PSUM)
                    nc.vector.tensor_mul(wns[j][:], wts[j][:], rb_b)
                wn_ready = j1
                emitted_group += 1

            # logits + exp for this super across all batch blocks
            ns = len(sup)
            for nb in range(NB):
                pm = psm.tile([128, SUP * CH], f32, tag="pm", name=f"pm{si}_{nb}")
                for ci, j in enumerate(sup):
                    for t in range(KB // 2):
                        nc.tensor.matmul(
                            pm[:, ci * CH : (ci + 1) * CH],
                            lhsT=xs[:, 2 * t : 2 * t + 2, nb * 128 : (nb + 1) * 128],
                            rhs=wns[j][:, 2 * t : 2 * t + 2, :],
                            start=(t == 0),
                            stop=(t == KB // 2 - 1),
                            perf_mode=DR,
                        )
                es = escrp.tile([128, SUP * CH], f8, tag="es", name=f"es{si}_{nb}")
                nc.scalar.activation(
                    es[:, : ns * CH],
                    pm[:, : ns * CH],
                    mybir.ActivationFunctionType.Exp,
                    bias=nshift_b[:],
                    scale=1.0,
                    accum_out=s_parts[:, nb * NSUP + si : nb * NSUP + si + 1],
                )

        s_fin = singles.tile([128, NB], f32)
        nc.vector.tensor_reduce(
            out=s_fin[:],
            in_=s_parts[:].rearrange("p (nb nsup) -> p nb nsup", nb=NB),
            axis=mybir.AxisListType.X,
            op=mybir.AluOpType.add,
        )
        nc.sync.dma_start(out=s_out.rearrange("nb p -> p nb"), in_=s_fin[:])

    nc.compile()
    return nc


def _get_nc():
    if "nc" not in _CACHE:
        _CACHE["nc"] = _build_nc()
    return _CACHE["nc"]


def _prep_inputs(x, weights):
    """Host-side shard/layout prep: normalize x (fp8, x8), shard+transpose W."""
    x = np.asarray(x, dtype=np.float32)
    w = np.asarray(weights, dtype=np.float32)

    xn = x / np.linalg.norm(x.astype(np.float64), axis=1, keepdims=True)
    xnt = np.ascontiguousarray(xn.T * F8SCALE).astype(ml_dtypes.float8_e4m3)

    wpad = np.zeros((CPAD, D), dtype=np.float32)
    wpad[:C] = w
    wt_maps = []
    for i in range(NCORES):
        shard = wpad[i * CSH : (i + 1) * CSH]  # [12800, 512]
        # -> [p, j, k, c] with [j,k,c] contiguous per partition
        arr = shard.reshape(NCH, CH, KB, 128).transpose(3, 0, 2, 1)
        wt_maps.append(np.ascontiguousarray(arr).astype(ml_dtypes.bfloat16))
    return xnt, wt_maps


def _run_on_device(xnt, wt_maps, trace=False):
    from concourse.bass_utils import run_bass_kernel_spmd

    nc = _get_nc()
    in_maps = [{"xnt": xnt, "wt": wt_maps[i]} for i in range(NCORES)]
    res = run_bass_kernel_spmd(
        nc, in_maps, core_ids=list(range(NCORES)), trace=trace
    )
    _CACHE["last_results"] = res
    return [r["s_out"].reshape(B).astype(np.float64) for r in res.results]


def kernel(x, weights, targets, _trace=False):
    x = np.asarray(x)
    weights = np.asarray(weights)
    targets = np.asarray(targets).astype(np.int64)

    xnt, wt_maps = _prep_inputs(x, weights)
    s_shards = _run_on_device(xnt, wt_maps, trace=_trace)

    # ---- host combine (f64, ~0.5 MFLOP total) ----
    s_total = np.sum(s_shards, axis=0)  # [B]
    # remove zero-pad classes: each contributes exp(0*S - SHIFT) exactly
    npad = CPAD - C
    s_total = s_total - npad * math.exp(-SHIFT)

    xf = x.astype(np.float64)
    xn = xf / np.linalg.norm(xf, axis=1, keepdims=True)
    wtg = weights.astype(np.float64)[targets]  # [B, D] gathered target rows
    wtg = wtg / np.linalg.norm(wtg, axis=1, keepdims=True)
    cos_t = np.einsum("bd,bd->b", xn, wtg)

    sin_t = np.sqrt(np.clip(1.0 - cos_t * cos_t, 0.0, 1.0))
    phi = cos_t * COS_M - sin_t * SIN_M
    psi = np.where(cos_t > TH, phi, cos_t - MM)

    # swap the target term: remove exp(S*cos_t), add exp(S*psi)
    s_adj = s_total - np.exp(S * cos_t - SHIFT) + np.exp(S * psi - SHIFT)
    lse = SHIFT + np.log(s_adj)
    loss = np.mean(lse - S * psi)
    return np.float32(loss)
